# revision 5
# baseline (speedup 1.0000x reference)
"""Trainium2 Bass kernel for nn_CentroidDistance (Poincare centroid distance).

Math (reference):
    sq    = max(||x||^2 + ||c||^2 - 2 x.c, 0)
    denom = max((1-||x||^2)(1-||c||^2), 1e-12)
    arg   = 1 + 2 sq / denom
    d     = arccosh(max(arg, 1+eps))
    node_centroid_dist  = d * mask            # [1, N, C]
    graph_centroid_dist = sum(d*mask) / sum(mask)   # [1, C]

Strategy: data-parallel over the node dimension across 8 NeuronCores.
Host folds the per-row factor a_i = mask_i/(1-sx_i) and per-column factor
b_j = 2/(1-sc_j) into an augmented bf16 GEMM so that the single device
GEMM produces w_ij = mask_i * 2*sq/denom directly in PSUM:

    xhat (stationary, [261, Npad] bf16):
        rows 0..255 : a_i * x_i  (transposed)
        row  256    : a_i
        rows 257-260: hi(a_i*sx_i), lo(a_i*sx_i), hi(a_i*sx_i), lo(a_i*sx_i)
    cmov (moving, [261, 256] bf16):
        rows 0..255 : -2 * b_j * c_j  (transposed)
        row  256    : b_j * sc_j
        rows 257-260: hi(b_j), hi(b_j), lo(b_j), lo(b_j)

(hi/lo bf16 splits keep the large ||x||^2 term at ~fp32 accuracy.)

Then arccosh(1+w) = ln(1 + w + sqrt(w(w+2))) is computed on-device:
    ACT: y = Square(w + 1)          (= (w+1)^2, table-set agnostic)
    ACT: s = Sqrt(y - 1)            (= sqrt(w(w+2)), sqrt table set)
    DVE: z = w + s                  (fused scalar_tensor_tensor from PSUM)
    ACT: d = Ln(z + 1)              (ln table set, phase-batched)
Masked rows have w == 0 exactly -> d == 0 exactly.
"""

import sys

for _p in ("/opt/trn_rl_repo",):
    if _p not in sys.path:
        sys.path.insert(0, _p)

import numpy as np
import ml_dtypes

import concourse.bass as bass
import concourse.tile as tile
from concourse import bacc, mybir
from concourse.bass_utils import run_bass_kernel_spmd


def _ensure_ntff_hook():
    """The agent image's `antenv` lacks `axon_hooks`; bass_utils hard-imports
    it for trace=True under axon. Shim the module and register the ctypes
    NTFF hook against the injected libaxon_pjrt.so."""
    import types
    try:
        import antenv.axon_hooks  # noqa: F401
        return
    except ImportError:
        pass
    import antenv
    mod = types.ModuleType("antenv.axon_hooks")
    mod._hook = None

    def set_axon_ntff_profile_hook(h):
        mod._hook = h

    def get_axon_ntff_profile_hook():
        return mod._hook

    mod.set_axon_ntff_profile_hook = set_axon_ntff_profile_hook
    mod.get_axon_ntff_profile_hook = get_axon_ntff_profile_hook
    sys.modules["antenv.axon_hooks"] = mod
    antenv.axon_hooks = mod

    so_path = "/opt/axon/libaxon_pjrt.so"
    try:
        from trn_agent_boot.trn_boot import _ntff_profile_via_ctypes
        hook = _ntff_profile_via_ctypes(so_path)
        if hook is not None:
            mod._hook = hook
    except Exception:
        pass


_ensure_ntff_hook()

BF16 = ml_dtypes.bfloat16

N = 150000
D = 256
C = 256
N_CORES = 8
N_PER = N // N_CORES          # 18750 nodes per core
TILE = 128                    # nodes per matmul tile
SUP = 7                       # node-tiles per super-tile
SUP_NODES = SUP * TILE        # 896 nodes per super-tile
NSUP = 21                     # super-tiles per core
N_PAD = NSUP * SUP_NODES      # 18816 padded nodes per core
FD = SUP * C                  # 1792 free-dim elements per super-tile
K = 261                       # 256 + 1 + 4 augmented contraction dim
GROUP = 7                     # super-tiles per ACT table-set phase group

_PROGRAM_CACHE = {}


def build_program(out_dtype=mybir.dt.float32, with_partial=False):
    """Build the per-core Bass program (identical for all 8 cores)."""
    nc = bacc.Bacc("TRN2", target_bir_lowering=False, debug=False,
                   enable_asserts=False)
    dt = mybir.dt

    xhat = nc.declare_dram_parameter("xhat", [K, N_PAD], dt.bfloat16,
                                     isOutput=False)
    cmov = nc.declare_dram_parameter("cmov", [K, C], dt.bfloat16,
                                     isOutput=False)
    out = nc.declare_dram_parameter("out", [N_PAD, C], out_dtype,
                                    isOutput=True)

    AF = mybir.ActivationFunctionType
    ALU = mybir.AluOpType

    # const AP for Sqrt's bias=-1.0 (only 0.0/1.0 pre-registered)
    _cm1 = nc.alloc_sbuf_tensor("const-f32-neg1", [128, 1], dt.float32)
    nc.gpsimd.memset(_cm1.ap(), -1.0)
    nc.const_aps.aps[(dt.float32, -1.0)] = _cm1.ap()
    nc.all_engine_barrier()

    KS = [(0, 128), (128, 128), (256, K - 256)]  # k-tile (start, size)

    with tile.TileContext(nc) as tc:
        import contextlib
        ctx = contextlib.ExitStack()
        with ctx:
            cpool = ctx.enter_context(tc.tile_pool(name="cmov", bufs=1))
            xpool = ctx.enter_context(tc.tile_pool(name="xhat", bufs=3))
            psum_pool = ctx.enter_context(
                tc.tile_pool(name="psum", bufs=2, space="PSUM"))
            ypool = ctx.enter_context(tc.tile_pool(name="y", bufs=3))
            spool = ctx.enter_context(tc.tile_pool(name="s", bufs=3))
            zpool = ctx.enter_context(tc.tile_pool(name="z", bufs=GROUP + 2))
            dpool = ctx.enter_context(tc.tile_pool(name="d", bufs=3))

            # load the replicated moving operand once
            cm = []
            for (k0, ksz) in KS:
                t = cpool.tile([ksz, C], dt.bfloat16, tag=f"cm{k0}")
                nc.sync.dma_start(t[:], cmov[k0:k0 + ksz, :])
                cm.append(t)

            n_groups = NSUP // GROUP
            for g in range(n_groups):
                zs = []
                for si in range(GROUP):
                    s_idx = g * GROUP + si
                    c0 = s_idx * SUP_NODES  # node column offset

                    xk = []
                    for (k0, ksz) in KS:
                        t = xpool.tile([ksz, SUP_NODES], dt.bfloat16,
                                       tag=f"xk{k0}")
                        nc.sync.dma_start(
                            t[:], xhat[k0:k0 + ksz, c0:c0 + SUP_NODES])
                        xk.append(t)

                    pw = psum_pool.tile([TILE, FD], dt.float32, tag="w")
                    for nt in range(SUP):
                        for ki in range(3):
                            nc.tensor.matmul(
                                pw[:, nt * C:(nt + 1) * C],
                                lhsT=xk[ki][:, nt * TILE:(nt + 1) * TILE],
                                rhs=cm[ki][:],
                                start=(ki == 0), stop=(ki == 2),
                            )

                    y = ypool.tile([TILE, FD], dt.float32, tag="y")
                    nc.scalar.activation(y[:], pw[:], AF.Square, bias=1.0)
                    s = spool.tile([TILE, FD], dt.float32, tag="s")
                    nc.scalar.activation(s[:], y[:], AF.Sqrt, bias=-1.0)
                    z = zpool.tile([TILE, FD], dt.float32, tag="z")
                    nc.vector.scalar_tensor_tensor(
                        z[:], in0=pw[:], scalar=0.0, in1=s[:],
                        op0=ALU.bypass, op1=ALU.add)
                    zs.append(z)

                # ln phase for the whole group (single table-set switch)
                for si in range(GROUP):
                    s_idx = g * GROUP + si
                    c0 = s_idx * SUP_NODES
                    d_t = dpool.tile([TILE, FD], out_dtype, tag="d")
                    nc.scalar.activation(d_t[:], zs[si][:], AF.Ln, bias=1.0)
                    dst = out[c0:c0 + SUP_NODES, :].rearrange(
                        "(nt p) c -> p nt c", p=TILE)
                    src = d_t[:].rearrange("p (nt c) -> p nt c", c=C)
                    nc.sync.dma_start(dst, src)

    nc.compile()
    return nc


def get_program(**kw):
    key = tuple(sorted(kw.items()))
    if key not in _PROGRAM_CACHE:
        _PROGRAM_CACHE[key] = build_program(**kw)
    return _PROGRAM_CACHE[key]


def _hi_lo(v):
    hi = v.astype(BF16)
    lo = (v - hi.astype(np.float32)).astype(BF16)
    return hi, lo


def host_prep(node_repr, mask, centroid_weight):
    """Build per-core xhat shards and the replicated cmov matrix."""
    x = np.ascontiguousarray(node_repr, dtype=np.float32)
    m = np.ascontiguousarray(mask, dtype=np.float32).reshape(-1)
    c = np.ascontiguousarray(centroid_weight, dtype=np.float32)

    sx = np.einsum("nd,nd->n", x, x, dtype=np.float32)
    sc = np.einsum("cd,cd->c", c, c, dtype=np.float32)
    a = m / (1.0 - sx)                      # mask folded in
    b = 2.0 / (1.0 - sc)

    # moving operand [K, C]
    cmov = np.zeros((K, C), dtype=BF16)
    cmov[0:D, :] = (-2.0 * b[:, None] * c).T.astype(BF16)
    cmov[D, :] = (b * sc).astype(BF16)
    bhi, blo = _hi_lo(b)
    cmov[D + 1, :] = bhi
    cmov[D + 2, :] = bhi
    cmov[D + 3, :] = blo
    cmov[D + 4, :] = blo

    v = a * sx
    vhi, vlo = _hi_lo(v)
    ax = (x * a[:, None]).astype(BF16)      # [N, D]
    abf = a.astype(BF16)

    xhats = []
    for i in range(N_CORES):
        n0, n1 = i * N_PER, (i + 1) * N_PER
        xh = np.zeros((K, N_PAD), dtype=BF16)
        xh[0:D, :N_PER] = ax[n0:n1].T
        xh[D, :N_PER] = abf[n0:n1]
        xh[D + 1, :N_PER] = vhi[n0:n1]
        xh[D + 2, :N_PER] = vlo[n0:n1]
        xh[D + 3, :N_PER] = vhi[n0:n1]
        xh[D + 4, :N_PER] = vlo[n0:n1]
        xhats.append(xh)
    return xhats, cmov, m


def kernel(node_repr, mask, centroid_weight, trace=False, out_dtype="float32"):
    xhats, cmov, m = host_prep(node_repr, mask, centroid_weight)

    odt = mybir.dt.float32 if out_dtype == "float32" else mybir.dt.float16
    nc = get_program(out_dtype=odt, with_partial=False)

    in_maps = [{"xhat": xhats[i], "cmov": cmov} for i in range(N_CORES)]
    res = run_bass_kernel_spmd(nc, in_maps, core_ids=list(range(N_CORES)),
                               trace=trace)

    parts = []
    for i in range(N_CORES):
        o = res.results[i]["out"]
        parts.append(np.asarray(o[:N_PER], dtype=np.float32))

    node_centroid_dist = np.concatenate(parts, axis=0)[None]  # [1, N, C]
    msum = m.sum(dtype=np.float32)
    graph_centroid_dist = (
        node_centroid_dist[0].sum(axis=0, dtype=np.float32) / msum)[None]
    if trace:
        kernel.last_result = res
    return graph_centroid_dist, node_centroid_dist


# revision 9
# speedup vs baseline: 1.0698x; 1.0698x over previous
"""Trainium2 Bass kernel for nn_CentroidDistance (Poincare centroid distance).

Math (reference):
    sq    = max(||x||^2 + ||c||^2 - 2 x.c, 0)
    denom = max((1-||x||^2)(1-||c||^2), 1e-12)
    arg   = 1 + 2 sq / denom
    d     = arccosh(max(arg, 1+eps))
    node_centroid_dist  = d * mask            # [1, N, C]
    graph_centroid_dist = sum(d*mask) / sum(mask)   # [1, C]

Strategy: data-parallel over the node dimension across 8 NeuronCores.
Host folds the per-row factor a_i = mask_i/(1-sx_i) and per-column factor
b_j = 2/(1-sc_j) into an augmented bf16 GEMM so that the single device
GEMM produces w_ij = mask_i * 2*sq/denom directly in PSUM:

    xhat (stationary, [261, Npad] bf16):
        rows 0..255 : a_i * x_i  (transposed)
        row  256    : a_i
        rows 257-260: hi(a_i*sx_i), lo(a_i*sx_i), hi(a_i*sx_i), lo(a_i*sx_i)
    cmov (moving, [261, 256] bf16):
        rows 0..255 : -2 * b_j * c_j  (transposed)
        row  256    : b_j * sc_j
        rows 257-260: hi(b_j), hi(b_j), lo(b_j), lo(b_j)

(hi/lo bf16 splits keep the large ||x||^2 term at ~fp32 accuracy.)

Then arccosh(1+w) = ln(1 + w + sqrt(w(w+2))) is computed on-device:
    ACT: y = Square(w + 1)          (= (w+1)^2, table-set agnostic)
    ACT: s = Sqrt(y - 1)            (= sqrt(w(w+2)), sqrt table set)
    DVE: z = w + s                  (fused scalar_tensor_tensor from PSUM)
    ACT: d = Ln(z + 1)              (ln table set, phase-batched)
Masked rows have w == 0 exactly -> d == 0 exactly.
"""

import sys

for _p in ("/opt/trn_rl_repo",):
    if _p not in sys.path:
        sys.path.insert(0, _p)

import numpy as np
import ml_dtypes

import concourse.bass as bass
import concourse.tile as tile
from concourse import bacc, mybir
from concourse.bass_utils import run_bass_kernel_spmd


def _ensure_ntff_hook():
    """The agent image's `antenv` lacks `axon_hooks`; bass_utils hard-imports
    it for trace=True under axon. Shim the module and register the ctypes
    NTFF hook against the injected libaxon_pjrt.so."""
    import types
    try:
        import antenv.axon_hooks  # noqa: F401
        return
    except ImportError:
        pass
    import antenv
    mod = types.ModuleType("antenv.axon_hooks")
    mod._hook = None

    def set_axon_ntff_profile_hook(h):
        mod._hook = h

    def get_axon_ntff_profile_hook():
        return mod._hook

    mod.set_axon_ntff_profile_hook = set_axon_ntff_profile_hook
    mod.get_axon_ntff_profile_hook = get_axon_ntff_profile_hook
    sys.modules["antenv.axon_hooks"] = mod
    antenv.axon_hooks = mod

    so_path = "/opt/axon/libaxon_pjrt.so"
    try:
        from trn_agent_boot.trn_boot import _ntff_profile_via_ctypes
        hook = _ntff_profile_via_ctypes(so_path)
        if hook is not None:
            mod._hook = hook
    except Exception:
        pass


_ensure_ntff_hook()

BF16 = ml_dtypes.bfloat16

N = 150000
D = 256
C = 256
N_CORES = 8
N_PER = N // N_CORES          # 18750 nodes per core
TILE = 128                    # nodes per matmul tile
SUP = 7                       # node-tiles per super-tile
SUP_NODES = SUP * TILE        # 896 nodes per super-tile
NSUP = 21                     # super-tiles per core
N_PAD = NSUP * SUP_NODES      # 18816 padded nodes per core
FD = SUP * C                  # 1792 free-dim elements per super-tile
K = 261                       # 256 + 1 + 4 augmented contraction dim
GROUP = 7                     # super-tiles per ACT table-set phase group

_PROGRAM_CACHE = {}


def build_program(out_dtype=mybir.dt.float32, y_on_dve=True):
    """Build the per-core Bass program (identical for all 8 cores).

    GEMM orientation: stationary = cmov halves (constant, so LDWEIGHTS
    amortizes), moving = xhat node columns.  PSUM gets w in [C, nodes]
    orientation; output DRAM is outT [C, N_PAD] and the host transposes
    during unshard.
    """
    from concourse.tile_rust import add_dep_helper

    nc = bacc.Bacc("TRN2", target_bir_lowering=False, debug=False,
                   enable_asserts=False)
    dt = mybir.dt

    xhat = nc.declare_dram_parameter("xhat", [K, N_PAD], dt.bfloat16,
                                     isOutput=False)
    cmov = nc.declare_dram_parameter("cmov", [K, C], dt.bfloat16,
                                     isOutput=False)
    outT = nc.declare_dram_parameter("outT", [C, N_PAD], out_dtype,
                                     isOutput=True)

    AF = mybir.ActivationFunctionType
    ALU = mybir.AluOpType

    # const AP for Sqrt's bias=-1.0 (only 0.0/1.0 pre-registered)
    _cm1 = nc.alloc_sbuf_tensor("const-f32-neg1", [128, 1], dt.float32)
    nc.gpsimd.memset(_cm1.ap(), -1.0)
    nc.const_aps.aps[(dt.float32, -1.0)] = _cm1.ap()
    nc.all_engine_barrier()

    KS = [(0, 128), (128, 128), (256, K - 256)]  # k-tile (start, size)
    SW = SUP_NODES            # 896 nodes per sweep
    NSW = NSUP                # 21 sweeps
    # psum layout [128, 2048]: ch0 nodes at [0:896], ch1 at [1024:1920]
    # (1024-offset keeps every matmul slice inside one 2KB bank)
    CH_OFF = (0, 1024)
    MM_SPLIT = ((0, 512), (512, 384))  # N<=512 fp32 psum-bank limit

    with tile.TileContext(nc) as tc:
        import contextlib
        ctx = contextlib.ExitStack()
        with ctx:
            cpool = ctx.enter_context(tc.tile_pool(name="cmov", bufs=1))
            xpool = ctx.enter_context(tc.tile_pool(name="xhat", bufs=3))
            psum_pool = ctx.enter_context(
                tc.tile_pool(name="psum", bufs=2, space="PSUM"))
            ypool = ctx.enter_context(tc.tile_pool(name="y", bufs=4))
            spool = ctx.enter_context(tc.tile_pool(name="s", bufs=4))
            zpool = ctx.enter_context(
                tc.tile_pool(name="z", bufs=2 * GROUP + 2))
            dpool = ctx.enter_context(tc.tile_pool(name="d", bufs=4))

            # load the replicated stationary operand once
            cm = []
            for (k0, ksz) in KS:
                t = cpool.tile([ksz, C], dt.bfloat16, tag=f"cm{k0}")
                nc.sync.dma_start(t[:], cmov[k0:k0 + ksz, :])
                cm.append(t)

            n_groups = NSW // GROUP
            last_d_inst = None
            for g in range(n_groups):
                zs = []
                last_s_inst = None
                first_s_inst = None
                for si in range(GROUP):
                    sw = g * GROUP + si
                    n0 = sw * SW

                    xk = []
                    for (k0, ksz) in KS:
                        t = xpool.tile([ksz, SW], dt.bfloat16, tag=f"xk{k0}")
                        nc.sync.dma_start(t[:], xhat[k0:k0 + ksz, n0:n0 + SW])
                        xk.append(t)

                    pw = psum_pool.tile([TILE, 2048], dt.float32, tag="w")
                    for ch in range(2):
                        for ki in range(3):
                            for (f0, fsz) in MM_SPLIT:
                                nc.tensor.matmul(
                                    pw[:, CH_OFF[ch] + f0:
                                       CH_OFF[ch] + f0 + fsz],
                                    lhsT=cm[ki][:, ch * 128:(ch + 1) * 128],
                                    rhs=xk[ki][:, f0:f0 + fsz],
                                    start=(ki == 0), stop=(ki == 2),
                                )

                    for ch in range(2):
                        pws = pw[:, CH_OFF[ch]:CH_OFF[ch] + SW]
                        z = zpool.tile([TILE, SW], dt.float32, tag="z")
                        s = spool.tile([TILE, SW], dt.float32, tag="s")
                        if ch == 0:
                            # ACT-heavy: y=(w+1)^2, s=sqrt(y-1) on ACT;
                            # z=w+s on DVE (PSUM read)
                            y = ypool.tile([TILE, SW], dt.float32, tag="y")
                            nc.scalar.activation(y[:], pws, AF.Square,
                                                 bias=1.0)
                            s_inst = nc.scalar.activation(
                                s[:], y[:], AF.Sqrt, bias=-1.0)
                            nc.vector.scalar_tensor_tensor(
                                z[:], in0=pws, scalar=0.0, in1=s[:],
                                op0=ALU.bypass, op1=ALU.add)
                        else:
                            # DVE-heavy: evac w, r=(w+2)*w, z=w+s on DVE;
                            # only sqrt on ACT
                            ws = ypool.tile([TILE, SW], dt.float32,
                                            tag="ws")
                            nc.vector.tensor_scalar(
                                ws[:], pws, 1.0, None, op0=ALU.mult)
                            r = ypool.tile([TILE, SW], dt.float32, tag="r")
                            nc.vector.scalar_tensor_tensor(
                                r[:], in0=ws[:], scalar=2.0, in1=ws[:],
                                op0=ALU.add, op1=ALU.mult)
                            s_inst = nc.scalar.activation(
                                s[:], r[:], AF.Sqrt)
                            nc.vector.scalar_tensor_tensor(
                                z[:], in0=ws[:], scalar=0.0, in1=s[:],
                                op0=ALU.bypass, op1=ALU.add)
                        if first_s_inst is None:
                            first_s_inst = s_inst
                        last_s_inst = s_inst
                        zs.append((z, sw, ch))

                # keep next group's sqrt-phase after this group's ln-phase
                if last_d_inst is not None and first_s_inst is not None:
                    add_dep_helper(first_s_inst.ins, last_d_inst.ins,
                                   sync=False,
                                   reason="ACT table-set phase order")

                first_d_inst = None
                for (z, sw, ch) in zs:
                    n0 = sw * SW
                    d_t = dpool.tile([TILE, SW], out_dtype, tag="d")
                    d_inst = nc.scalar.activation(d_t[:], z[:], AF.Ln,
                                                  bias=1.0)
                    if first_d_inst is None:
                        # ln-phase starts only after the whole sqrt-phase
                        add_dep_helper(d_inst.ins, last_s_inst.ins,
                                       sync=False,
                                       reason="ACT table-set phase order")
                        first_d_inst = d_inst
                    last_d_inst = d_inst
                    nc.sync.dma_start(
                        outT[ch * 128:(ch + 1) * 128, n0:n0 + SW], d_t[:])

    nc.compile()
    return nc


def get_program(**kw):
    key = tuple(sorted(kw.items()))
    if key not in _PROGRAM_CACHE:
        _PROGRAM_CACHE[key] = build_program(**kw)
    return _PROGRAM_CACHE[key]


Y_ON_DVE = True


def _hi_lo(v):
    hi = v.astype(BF16)
    lo = (v - hi.astype(np.float32)).astype(BF16)
    return hi, lo


def host_prep(node_repr, mask, centroid_weight):
    """Build per-core xhat shards and the replicated cmov matrix."""
    x = np.ascontiguousarray(node_repr, dtype=np.float32)
    m = np.ascontiguousarray(mask, dtype=np.float32).reshape(-1)
    c = np.ascontiguousarray(centroid_weight, dtype=np.float32)

    sx = np.einsum("nd,nd->n", x, x, dtype=np.float32)
    sc = np.einsum("cd,cd->c", c, c, dtype=np.float32)
    a = m / (1.0 - sx)                      # mask folded in
    b = 2.0 / (1.0 - sc)

    # moving operand [K, C]
    cmov = np.zeros((K, C), dtype=BF16)
    cmov[0:D, :] = (-2.0 * b[:, None] * c).T.astype(BF16)
    cmov[D, :] = (b * sc).astype(BF16)
    bhi, blo = _hi_lo(b)
    cmov[D + 1, :] = bhi
    cmov[D + 2, :] = bhi
    cmov[D + 3, :] = blo
    cmov[D + 4, :] = blo

    v = a * sx
    vhi, vlo = _hi_lo(v)
    ax = (x * a[:, None]).astype(BF16)      # [N, D]
    abf = a.astype(BF16)

    xhats = []
    for i in range(N_CORES):
        n0, n1 = i * N_PER, (i + 1) * N_PER
        xh = np.zeros((K, N_PAD), dtype=BF16)
        xh[0:D, :N_PER] = ax[n0:n1].T
        xh[D, :N_PER] = abf[n0:n1]
        xh[D + 1, :N_PER] = vhi[n0:n1]
        xh[D + 2, :N_PER] = vlo[n0:n1]
        xh[D + 3, :N_PER] = vhi[n0:n1]
        xh[D + 4, :N_PER] = vlo[n0:n1]
        xhats.append(xh)
    return xhats, cmov, m


def kernel(node_repr, mask, centroid_weight, trace=False, out_dtype="float32"):
    xhats, cmov, m = host_prep(node_repr, mask, centroid_weight)

    odt = mybir.dt.float32 if out_dtype == "float32" else mybir.dt.float16
    nc = get_program(out_dtype=odt, y_on_dve=Y_ON_DVE)

    in_maps = [{"xhat": xhats[i], "cmov": cmov} for i in range(N_CORES)]
    res = run_bass_kernel_spmd(nc, in_maps, core_ids=list(range(N_CORES)),
                               trace=trace)

    parts = []
    gsum = np.zeros((C,), dtype=np.float32)
    for i in range(N_CORES):
        o = np.asarray(res.results[i]["outT"][:, :N_PER], dtype=np.float32)
        gsum += o.sum(axis=1, dtype=np.float32)
        parts.append(o.T)

    node_centroid_dist = np.ascontiguousarray(
        np.concatenate(parts, axis=0))[None]  # [1, N, C]
    msum = m.sum(dtype=np.float32)
    graph_centroid_dist = (gsum / msum)[None]
    if trace:
        kernel.last_result = res
    return graph_centroid_dist, node_centroid_dist


# revision 12
# speedup vs baseline: 1.3707x; 1.2812x over previous
"""Trainium2 Bass kernel for nn_CentroidDistance (Poincare centroid distance).

Math (reference):
    sq    = max(||x||^2 + ||c||^2 - 2 x.c, 0)
    denom = max((1-||x||^2)(1-||c||^2), 1e-12)
    arg   = 1 + 2 sq / denom
    d     = arccosh(max(arg, 1+eps))
    node_centroid_dist  = d * mask            # [1, N, C]
    graph_centroid_dist = sum(d*mask) / sum(mask)   # [1, C]

Strategy: data-parallel over the node dimension across 8 NeuronCores.
Host folds the per-row factor a_i = mask_i/(1-sx_i) and per-column factor
b_j = 2/(1-sc_j) into an augmented bf16 GEMM so that the single device
GEMM produces w_ij = mask_i * 2*sq/denom directly in PSUM:

    xhat (stationary, [261, Npad] bf16):
        rows 0..255 : a_i * x_i  (transposed)
        row  256    : a_i
        rows 257-260: hi(a_i*sx_i), lo(a_i*sx_i), hi(a_i*sx_i), lo(a_i*sx_i)
    cmov (moving, [261, 256] bf16):
        rows 0..255 : -2 * b_j * c_j  (transposed)
        row  256    : b_j * sc_j
        rows 257-260: hi(b_j), hi(b_j), lo(b_j), lo(b_j)

(hi/lo bf16 splits keep the large ||x||^2 term at ~fp32 accuracy.)

Then arccosh(1+w) = ln(1 + w + sqrt(w(w+2))) is computed on-device:
    ACT: y = Square(w + 1)          (= (w+1)^2, table-set agnostic)
    ACT: s = Sqrt(y - 1)            (= sqrt(w(w+2)), sqrt table set)
    DVE: z = w + s                  (fused scalar_tensor_tensor from PSUM)
    ACT: d = Ln(z + 1)              (ln table set, phase-batched)
Masked rows have w == 0 exactly -> d == 0 exactly.
"""

import sys

for _p in ("/opt/trn_rl_repo",):
    if _p not in sys.path:
        sys.path.insert(0, _p)

import numpy as np
import ml_dtypes

import concourse.bass as bass
import concourse.tile as tile
from concourse import bacc, mybir
from concourse.bass_utils import run_bass_kernel_spmd


def _ensure_ntff_hook():
    """The agent image's `antenv` lacks `axon_hooks`; bass_utils hard-imports
    it for trace=True under axon. Shim the module and register the ctypes
    NTFF hook against the injected libaxon_pjrt.so."""
    import types
    try:
        import antenv.axon_hooks  # noqa: F401
        return
    except ImportError:
        pass
    import antenv
    mod = types.ModuleType("antenv.axon_hooks")
    mod._hook = None

    def set_axon_ntff_profile_hook(h):
        mod._hook = h

    def get_axon_ntff_profile_hook():
        return mod._hook

    mod.set_axon_ntff_profile_hook = set_axon_ntff_profile_hook
    mod.get_axon_ntff_profile_hook = get_axon_ntff_profile_hook
    sys.modules["antenv.axon_hooks"] = mod
    antenv.axon_hooks = mod

    so_path = "/opt/axon/libaxon_pjrt.so"
    try:
        from trn_agent_boot.trn_boot import _ntff_profile_via_ctypes
        hook = _ntff_profile_via_ctypes(so_path)
        if hook is not None:
            mod._hook = hook
    except Exception:
        pass


_ensure_ntff_hook()

BF16 = ml_dtypes.bfloat16

N = 150000
D = 256
C = 256
N_CORES = 8
N_PER = N // N_CORES          # 18750 nodes per core
TILE = 128                    # nodes per matmul tile
SUP = 7                       # node-tiles per super-tile
SUP_NODES = SUP * TILE        # 896 nodes per super-tile
NSUP = 21                     # super-tiles per core
N_PAD = NSUP * SUP_NODES      # 18816 padded nodes per core
FD = SUP * C                  # 1792 free-dim elements per super-tile
K = 261                       # 256 + 1 + 4 augmented contraction dim
GROUP = 7                     # super-tiles per ACT table-set phase group

_PROGRAM_CACHE = {}


def build_program(out_dtype=mybir.dt.float32, y_on_dve=True):
    """Build the per-core Bass program (identical for all 8 cores).

    GEMM orientation: stationary = cmov halves (constant, so LDWEIGHTS
    amortizes), moving = xhat node columns.  PSUM gets w in [C, nodes]
    orientation; output DRAM is outT [C, N_PAD] and the host transposes
    during unshard.
    """
    from concourse.tile_rust import add_dep_helper

    nc = bacc.Bacc("TRN2", target_bir_lowering=False, debug=False,
                   enable_asserts=False)
    dt = mybir.dt

    xhat = nc.declare_dram_parameter("xhat", [K, N_PAD], dt.bfloat16,
                                     isOutput=False)
    cmov = nc.declare_dram_parameter("cmov", [K, C], dt.bfloat16,
                                     isOutput=False)
    outT = nc.declare_dram_parameter("outT", [C, N_PAD], out_dtype,
                                     isOutput=True)

    AF = mybir.ActivationFunctionType
    ALU = mybir.AluOpType

    # const AP for Sqrt's bias=-1.0 (only 0.0/1.0 pre-registered)
    _cm1 = nc.alloc_sbuf_tensor("const-f32-neg1", [128, 1], dt.float32)
    nc.gpsimd.memset(_cm1.ap(), -1.0)
    nc.const_aps.aps[(dt.float32, -1.0)] = _cm1.ap()
    nc.all_engine_barrier()

    KS = [(0, 128), (128, 128), (256, K - 256)]  # k-tile (start, size)
    SW = SUP_NODES            # 896 nodes per sweep
    NSW = NSUP                # 21 sweeps
    MM_SPLIT = ((0, 512), (512, 384))  # N<=512 fp32 psum-bank limit

    with tile.TileContext(nc) as tc:
        import contextlib
        ctx = contextlib.ExitStack()
        with ctx:
            cpool = ctx.enter_context(tc.tile_pool(name="cmov", bufs=1))
            xpool = ctx.enter_context(tc.tile_pool(name="xhat", bufs=6))
            psum_pool = ctx.enter_context(
                tc.tile_pool(name="psum", bufs=4, space="PSUM"))
            wpool = ctx.enter_context(
                tc.tile_pool(name="ws", bufs=2 * GROUP + 2))
            rpool = ctx.enter_context(tc.tile_pool(name="r", bufs=6))
            spool = ctx.enter_context(tc.tile_pool(name="s", bufs=6))
            zpool = ctx.enter_context(
                tc.tile_pool(name="z", bufs=2 * GROUP + 2))
            dpool = ctx.enter_context(tc.tile_pool(name="d", bufs=4))

            # load the replicated stationary operand once
            cm = []
            for (k0, ksz) in KS:
                t = cpool.tile([ksz, C], dt.bfloat16, tag=f"cm{k0}")
                nc.sync.dma_start(t[:], cmov[k0:k0 + ksz, :])
                cm.append(t)

            n_groups = NSW // GROUP
            last_d_inst = None
            for g in range(n_groups):
                # --- matmul + psum-evac + r stage (phase-free) ---
                pend = []  # (ws, r, sw, ch)
                for si in range(GROUP):
                    sw = g * GROUP + si
                    n0 = sw * SW

                    xk = []
                    for (k0, ksz) in KS:
                        t = xpool.tile([ksz, SW], dt.bfloat16, tag=f"xk{k0}")
                        nc.sync.dma_start(t[:], xhat[k0:k0 + ksz, n0:n0 + SW])
                        xk.append(t)

                    for ch in range(2):
                        pw = psum_pool.tile([TILE, 1024], dt.float32,
                                            tag="w")
                        for ki in range(3):
                            for (f0, fsz) in MM_SPLIT:
                                nc.tensor.matmul(
                                    pw[:, f0:f0 + fsz],
                                    lhsT=cm[ki][:, ch * 128:(ch + 1) * 128],
                                    rhs=xk[ki][:, f0:f0 + fsz],
                                    start=(ki == 0), stop=(ki == 2),
                                )
                        pws = pw[:, 0:SW]
                        # evac w to SBUF bf16 (frees psum, never phase-gated)
                        ws = wpool.tile([TILE, SW], dt.float16, tag="ws")
                        nc.vector.tensor_scalar(
                            ws[:], pws, 1.0, None, op0=ALU.mult)
                        # r = (w+2)*w at bf16 2x
                        r = rpool.tile([TILE, SW], dt.float16, tag="r")
                        nc.vector.scalar_tensor_tensor(
                            r[:], in0=ws[:], scalar=2.0, in1=ws[:],
                            op0=ALU.add, op1=ALU.mult)
                        pend.append((ws, r, sw, ch))

                # --- sqrt phase (ACT) + z on DVE right behind ---
                first_s_inst = None
                last_s_inst = None
                zs = []
                for (ws, r, sw, ch) in pend:
                    s = spool.tile([TILE, SW], dt.float16, tag="s")
                    s_inst = nc.scalar.activation(s[:], r[:], AF.Sqrt)
                    if first_s_inst is None:
                        first_s_inst = s_inst
                        # keep this group's sqrt-phase after the previous
                        # group's ln-phase (table-set discipline)
                        if last_d_inst is not None:
                            add_dep_helper(s_inst.ins, last_d_inst.ins,
                                           sync=False,
                                           reason="ACT table phase order")
                    last_s_inst = s_inst
                    z = zpool.tile([TILE, SW], dt.float16, tag="z")
                    nc.vector.scalar_tensor_tensor(
                        z[:], in0=ws[:], scalar=0.0, in1=s[:],
                        op0=ALU.bypass, op1=ALU.add)
                    zs.append((z, sw, ch))

                # --- ln phase (ACT) + store ---
                first_d_inst = None
                for (z, sw, ch) in zs:
                    n0 = sw * SW
                    d_t = dpool.tile([TILE, SW], out_dtype, tag="d")
                    d_inst = nc.scalar.activation(d_t[:], z[:], AF.Ln,
                                                  bias=1.0)
                    if first_d_inst is None:
                        add_dep_helper(d_inst.ins, last_s_inst.ins,
                                       sync=False,
                                       reason="ACT table phase order")
                        first_d_inst = d_inst
                    last_d_inst = d_inst
                    nc.sync.dma_start(
                        outT[ch * 128:(ch + 1) * 128, n0:n0 + SW], d_t[:])

    nc.compile()
    return nc


def get_program(**kw):
    key = tuple(sorted(kw.items()))
    if key not in _PROGRAM_CACHE:
        _PROGRAM_CACHE[key] = build_program(**kw)
    return _PROGRAM_CACHE[key]


Y_ON_DVE = True


def _hi_lo(v):
    hi = v.astype(BF16)
    lo = (v - hi.astype(np.float32)).astype(BF16)
    return hi, lo


def host_prep(node_repr, mask, centroid_weight):
    """Build per-core xhat shards and the replicated cmov matrix."""
    x = np.ascontiguousarray(node_repr, dtype=np.float32)
    m = np.ascontiguousarray(mask, dtype=np.float32).reshape(-1)
    c = np.ascontiguousarray(centroid_weight, dtype=np.float32)

    sx = np.einsum("nd,nd->n", x, x, dtype=np.float32)
    sc = np.einsum("cd,cd->c", c, c, dtype=np.float32)
    a = m / (1.0 - sx)                      # mask folded in
    b = 2.0 / (1.0 - sc)

    # moving operand [K, C]
    cmov = np.zeros((K, C), dtype=BF16)
    cmov[0:D, :] = (-2.0 * b[:, None] * c).T.astype(BF16)
    cmov[D, :] = (b * sc).astype(BF16)
    bhi, blo = _hi_lo(b)
    cmov[D + 1, :] = bhi
    cmov[D + 2, :] = bhi
    cmov[D + 3, :] = blo
    cmov[D + 4, :] = blo

    v = a * sx
    vhi, vlo = _hi_lo(v)
    ax = (x * a[:, None]).astype(BF16)      # [N, D]
    abf = a.astype(BF16)

    xhats = []
    for i in range(N_CORES):
        n0, n1 = i * N_PER, (i + 1) * N_PER
        xh = np.zeros((K, N_PAD), dtype=BF16)
        xh[0:D, :N_PER] = ax[n0:n1].T
        xh[D, :N_PER] = abf[n0:n1]
        xh[D + 1, :N_PER] = vhi[n0:n1]
        xh[D + 2, :N_PER] = vlo[n0:n1]
        xh[D + 3, :N_PER] = vhi[n0:n1]
        xh[D + 4, :N_PER] = vlo[n0:n1]
        xhats.append(xh)
    return xhats, cmov, m


def kernel(node_repr, mask, centroid_weight, trace=False, out_dtype="float16"):
    xhats, cmov, m = host_prep(node_repr, mask, centroid_weight)

    odt = mybir.dt.float32 if out_dtype == "float32" else mybir.dt.float16
    nc = get_program(out_dtype=odt)

    in_maps = [{"xhat": xhats[i], "cmov": cmov} for i in range(N_CORES)]
    res = run_bass_kernel_spmd(nc, in_maps, core_ids=list(range(N_CORES)),
                               trace=trace)

    parts = []
    gsum = np.zeros((C,), dtype=np.float32)
    for i in range(N_CORES):
        o = np.asarray(res.results[i]["outT"][:, :N_PER], dtype=np.float32)
        gsum += o.sum(axis=1, dtype=np.float32)
        parts.append(o.T)

    node_centroid_dist = np.ascontiguousarray(
        np.concatenate(parts, axis=0))[None]  # [1, N, C]
    msum = m.sum(dtype=np.float32)
    graph_centroid_dist = (gsum / msum)[None]
    if trace:
        kernel.last_result = res
    return graph_centroid_dist, node_centroid_dist


# revision 16
# speedup vs baseline: 1.5828x; 1.1548x over previous
"""Trainium2 Bass kernel for nn_CentroidDistance (Poincare centroid distance).

Math (reference):
    sq    = max(||x||^2 + ||c||^2 - 2 x.c, 0)
    denom = max((1-||x||^2)(1-||c||^2), 1e-12)
    arg   = 1 + 2 sq / denom
    d     = arccosh(max(arg, 1+eps))
    node_centroid_dist  = d * mask            # [1, N, C]
    graph_centroid_dist = sum(d*mask) / sum(mask)   # [1, C]

Strategy: data-parallel over the node dimension across 8 NeuronCores.
Host folds the per-row factor a_i = mask_i/(1-sx_i) and per-column factor
b_j = 2/(1-sc_j) into an augmented bf16 GEMM so that the single device
GEMM produces w_ij = mask_i * 2*sq/denom directly in PSUM:

    xhat (stationary, [261, Npad] bf16):
        rows 0..255 : a_i * x_i  (transposed)
        row  256    : a_i
        rows 257-260: hi(a_i*sx_i), lo(a_i*sx_i), hi(a_i*sx_i), lo(a_i*sx_i)
    cmov (moving, [261, 256] bf16):
        rows 0..255 : -2 * b_j * c_j  (transposed)
        row  256    : b_j * sc_j
        rows 257-260: hi(b_j), hi(b_j), lo(b_j), lo(b_j)

(hi/lo bf16 splits keep the large ||x||^2 term at ~fp32 accuracy.)

Then arccosh(1+w) = ln(1 + w + sqrt(w(w+2))) is computed on-device:
    ACT: y = Square(w + 1)          (= (w+1)^2, table-set agnostic)
    ACT: s = Sqrt(y - 1)            (= sqrt(w(w+2)), sqrt table set)
    DVE: z = w + s                  (fused scalar_tensor_tensor from PSUM)
    ACT: d = Ln(z + 1)              (ln table set, phase-batched)
Masked rows have w == 0 exactly -> d == 0 exactly.
"""

import sys

for _p in ("/opt/trn_rl_repo",):
    if _p not in sys.path:
        sys.path.insert(0, _p)

import numpy as np
import ml_dtypes

import concourse.bass as bass
import concourse.tile as tile
from concourse import bacc, mybir
from concourse.bass_utils import run_bass_kernel_spmd


def _ensure_ntff_hook():
    """The agent image's `antenv` lacks `axon_hooks`; bass_utils hard-imports
    it for trace=True under axon. Shim the module and register the ctypes
    NTFF hook against the injected libaxon_pjrt.so."""
    import types
    try:
        import antenv.axon_hooks  # noqa: F401
        return
    except ImportError:
        pass
    import antenv
    mod = types.ModuleType("antenv.axon_hooks")
    mod._hook = None

    def set_axon_ntff_profile_hook(h):
        mod._hook = h

    def get_axon_ntff_profile_hook():
        return mod._hook

    mod.set_axon_ntff_profile_hook = set_axon_ntff_profile_hook
    mod.get_axon_ntff_profile_hook = get_axon_ntff_profile_hook
    sys.modules["antenv.axon_hooks"] = mod
    antenv.axon_hooks = mod

    so_path = "/opt/axon/libaxon_pjrt.so"
    try:
        from trn_agent_boot.trn_boot import _ntff_profile_via_ctypes
        hook = _ntff_profile_via_ctypes(so_path)
        if hook is not None:
            mod._hook = hook
    except Exception:
        pass


_ensure_ntff_hook()

BF16 = ml_dtypes.bfloat16

N = 150000
D = 256
C = 256
N_CORES = 8
N_PER = N // N_CORES          # 18750 nodes per core
TILE = 128                    # nodes per matmul tile
SUP = 7                       # node-tiles per super-tile
SUP_NODES = SUP * TILE        # 896 nodes per super-tile
NSUP = 21                     # super-tiles per core
N_PAD = NSUP * SUP_NODES      # 18816 padded nodes per core
FD = SUP * C                  # 1792 free-dim elements per super-tile
K = 261                       # 256 + 1 + 4 augmented contraction dim
GROUP = 7                     # super-tiles per ACT table-set phase group

_PROGRAM_CACHE = {}


def build_program(out_dtype=mybir.dt.float32, y_on_dve=True):
    """Build the per-core Bass program (identical for all 8 cores).

    GEMM orientation: stationary = cmov halves (constant, so LDWEIGHTS
    amortizes), moving = xhat node columns.  PSUM gets w in [C, nodes]
    orientation; output DRAM is outT [C, N_PAD] and the host transposes
    during unshard.
    """
    from concourse.tile_rust import add_dep_helper

    nc = bacc.Bacc("TRN2", target_bir_lowering=False, debug=False,
                   enable_asserts=False)
    dt = mybir.dt

    xhat = nc.declare_dram_parameter("xhat", [K, N_PAD], dt.bfloat16,
                                     isOutput=False)
    cmov = nc.declare_dram_parameter("cmov", [K, C], dt.bfloat16,
                                     isOutput=False)
    outT = nc.declare_dram_parameter("outT", [C, N_PAD], out_dtype,
                                     isOutput=True)
    # tiny dummy output keeping the PE warm-up burst alive through DCE
    warm = nc.declare_dram_parameter("warm", [128, 1], dt.float32,
                                     isOutput=True)

    AF = mybir.ActivationFunctionType
    ALU = mybir.AluOpType

    # const AP for Sqrt's bias=-1.0 (only 0.0/1.0 pre-registered)
    _cm1 = nc.alloc_sbuf_tensor("const-f32-neg1", [128, 1], dt.float32)
    nc.gpsimd.memset(_cm1.ap(), -1.0)
    nc.const_aps.aps[(dt.float32, -1.0)] = _cm1.ap()
    nc.all_engine_barrier()

    KS = [(0, 128), (128, 128), (256, K - 256)]  # k-tile (start, size)
    SW = SUP_NODES            # 896 nodes per sweep
    NSW = NSUP                # 21 sweeps
    MM_SPLIT = ((0, 512), (512, 384))  # N<=512 fp32 psum-bank limit

    with tile.TileContext(nc) as tc:
        import contextlib
        ctx = contextlib.ExitStack()
        with ctx:
            cpool = ctx.enter_context(tc.tile_pool(name="cmov", bufs=1))
            xpool = ctx.enter_context(tc.tile_pool(name="xhat", bufs=6))
            psum_pool = ctx.enter_context(
                tc.tile_pool(name="psum", bufs=4, space="PSUM"))
            wpool = ctx.enter_context(
                tc.tile_pool(name="ws", bufs=GROUP + 2))
            rpool = ctx.enter_context(tc.tile_pool(name="r", bufs=6))
            spool = ctx.enter_context(tc.tile_pool(name="s", bufs=6))
            zpool = ctx.enter_context(
                tc.tile_pool(name="z", bufs=GROUP + 2))
            dpool = ctx.enter_context(tc.tile_pool(name="d", bufs=4))

            # load the replicated stationary operand once
            cm = []
            for (k0, ksz) in KS:
                t = cpool.tile([ksz, C], dt.bfloat16, tag=f"cm{k0}")
                nc.sync.dma_start(t[:], cmov[k0:k0 + ksz, :])
                cm.append(t)

            # PE warm-up burst: ~16 dense matmuls get HAM to K=8/8 before
            # the steady-state loop (whose small gaps never re-warm it)
            pwarm = psum_pool.tile([TILE, 1024], dt.float32, tag="w")
            for i in range(16):
                nc.tensor.matmul(pwarm[:, 0:256], lhsT=cm[0][:, 0:128],
                                 rhs=cm[0][:, 0:C], start=True, stop=True)
            wtile = dpool.tile([TILE, 1], dt.float32, tag="warmout")
            nc.vector.tensor_scalar(wtile[:], pwarm[:, 0:1], 1.0, None,
                                    op0=ALU.mult)
            nc.sync.dma_start(warm[:, :], wtile[:])

            n_groups = NSW // GROUP
            last_d_inst = None
            for g in range(n_groups):
                # --- matmul + psum-evac + y stage (phase-free) ---
                pend = []  # (u, y, sw)
                for si in range(GROUP):
                    sw = g * GROUP + si
                    n0 = sw * SW

                    xk = []
                    for (k0, ksz) in KS:
                        t = xpool.tile([ksz, SW], dt.bfloat16, tag=f"xk{k0}")
                        nc.sync.dma_start(t[:], xhat[k0:k0 + ksz, n0:n0 + SW])
                        xk.append(t)

                    u = wpool.tile([TILE, 2 * SW], dt.float16, tag="u")
                    for ch in range(2):
                        pw = psum_pool.tile([TILE, 1024], dt.float32,
                                            tag="w")
                        for ki in range(3):
                            for (f0, fsz) in MM_SPLIT:
                                nc.tensor.matmul(
                                    pw[:, f0:f0 + fsz],
                                    lhsT=cm[ki][:, ch * 128:(ch + 1) * 128],
                                    rhs=xk[ki][:, f0:f0 + fsz],
                                    start=(ki == 0), stop=(ki == 2),
                                )
                        # evac u = w+1 to SBUF fp16 (frees psum fast)
                        nc.vector.tensor_scalar(
                            u[:, ch * SW:(ch + 1) * SW], pw[:, 0:SW],
                            1.0, None, op0=ALU.add)
                    # y = u*u = (w+1)^2, fp16 TT at 2x
                    y = rpool.tile([TILE, 2 * SW], dt.float16, tag="y")
                    nc.vector.tensor_mul(y[:], u[:], u[:])
                    pend.append((u, y, sw))

                # --- sqrt phase (ACT) + z on DVE right behind ---
                first_s_inst = None
                last_s_inst = None
                zs = []
                for (u, y, sw) in pend:
                    s = spool.tile([TILE, 2 * SW], dt.float16, tag="s")
                    s_inst = nc.scalar.activation(s[:], y[:], AF.Sqrt,
                                                  bias=-1.0)
                    if first_s_inst is None:
                        first_s_inst = s_inst
                        # keep this group's sqrt-phase after the previous
                        # group's ln-phase (table-set discipline)
                        if last_d_inst is not None:
                            add_dep_helper(s_inst.ins, last_d_inst.ins,
                                           sync=False,
                                           reason="ACT table phase order")
                    last_s_inst = s_inst
                    # z = u + s = 1 + w + sqrt(w(w+2)); d = Ln(z) directly
                    z = zpool.tile([TILE, 2 * SW], dt.float16, tag="z")
                    nc.vector.tensor_add(z[:], u[:], s[:])
                    zs.append((z, sw))

                # --- ln phase (ACT) + store ---
                first_d_inst = None
                for (z, sw) in zs:
                    n0 = sw * SW
                    d_t = dpool.tile([TILE, 2 * SW], out_dtype, tag="d")
                    d_inst = nc.scalar.activation(d_t[:], z[:], AF.Ln)
                    if first_d_inst is None:
                        add_dep_helper(d_inst.ins, last_s_inst.ins,
                                       sync=False,
                                       reason="ACT table phase order")
                        first_d_inst = d_inst
                    last_d_inst = d_inst
                    for ch in range(2):
                        nc.sync.dma_start(
                            outT[ch * 128:(ch + 1) * 128, n0:n0 + SW],
                            d_t[:, ch * SW:(ch + 1) * SW])

    nc.compile()
    return nc


def get_program(**kw):
    key = tuple(sorted(kw.items()))
    if key not in _PROGRAM_CACHE:
        _PROGRAM_CACHE[key] = build_program(**kw)
    return _PROGRAM_CACHE[key]


Y_ON_DVE = True


def _hi_lo(v):
    hi = v.astype(BF16)
    lo = (v - hi.astype(np.float32)).astype(BF16)
    return hi, lo


def host_prep(node_repr, mask, centroid_weight):
    """Build per-core xhat shards and the replicated cmov matrix."""
    x = np.ascontiguousarray(node_repr, dtype=np.float32)
    m = np.ascontiguousarray(mask, dtype=np.float32).reshape(-1)
    c = np.ascontiguousarray(centroid_weight, dtype=np.float32)

    sx = np.einsum("nd,nd->n", x, x, dtype=np.float32)
    sc = np.einsum("cd,cd->c", c, c, dtype=np.float32)
    a = m / (1.0 - sx)                      # mask folded in
    b = 2.0 / (1.0 - sc)

    # moving operand [K, C]
    cmov = np.zeros((K, C), dtype=BF16)
    cmov[0:D, :] = (-2.0 * b[:, None] * c).T.astype(BF16)
    cmov[D, :] = (b * sc).astype(BF16)
    bhi, blo = _hi_lo(b)
    cmov[D + 1, :] = bhi
    cmov[D + 2, :] = bhi
    cmov[D + 3, :] = blo
    cmov[D + 4, :] = blo

    v = a * sx
    vhi, vlo = _hi_lo(v)
    ax = (x * a[:, None]).astype(BF16)      # [N, D]
    abf = a.astype(BF16)

    xhats = []
    for i in range(N_CORES):
        n0, n1 = i * N_PER, (i + 1) * N_PER
        xh = np.zeros((K, N_PAD), dtype=BF16)
        xh[0:D, :N_PER] = ax[n0:n1].T
        xh[D, :N_PER] = abf[n0:n1]
        xh[D + 1, :N_PER] = vhi[n0:n1]
        xh[D + 2, :N_PER] = vlo[n0:n1]
        xh[D + 3, :N_PER] = vhi[n0:n1]
        xh[D + 4, :N_PER] = vlo[n0:n1]
        xhats.append(xh)
    return xhats, cmov, m


def kernel(node_repr, mask, centroid_weight, trace=False, out_dtype="float16"):
    xhats, cmov, m = host_prep(node_repr, mask, centroid_weight)

    odt = mybir.dt.float32 if out_dtype == "float32" else mybir.dt.float16
    nc = get_program(out_dtype=odt)

    in_maps = [{"xhat": xhats[i], "cmov": cmov} for i in range(N_CORES)]
    res = run_bass_kernel_spmd(nc, in_maps, core_ids=list(range(N_CORES)),
                               trace=trace)

    parts = []
    gsum = np.zeros((C,), dtype=np.float32)
    for i in range(N_CORES):
        o = np.asarray(res.results[i]["outT"][:, :N_PER], dtype=np.float32)
        gsum += o.sum(axis=1, dtype=np.float32)
        parts.append(o.T)

    node_centroid_dist = np.ascontiguousarray(
        np.concatenate(parts, axis=0))[None]  # [1, N, C]
    msum = m.sum(dtype=np.float32)
    graph_centroid_dist = (gsum / msum)[None]
    if trace:
        kernel.last_result = res
    return graph_centroid_dist, node_centroid_dist
